# revision 2
# baseline (speedup 1.0000x reference)
"""Trainium2 Bass kernel for nn_LuongAttention.

Reference math (per batch b):
    S   = Dec @ Enc^T          # [T_dec, T_enc]
    Out = S @ Enc              # [T_dec, D]

By associativity:  Out = Dec @ (Enc^T @ Enc) = Dec @ G with G = Enc^T Enc
a [D, D] = [128, 128] Gram matrix.  This removes the [2048, 2048]
intermediate entirely and makes the kernel memory-bound.

Sharding: data-parallel over batch B=8 -> one batch per NeuronCore.

Numerics: inputs are fed as fp8 e3m4 (1-3-4).  Enc quantization error
averages out inside the 2048-term Gram sums; Dec error passes through
elementwise (output is dominated by the G diagonal) but e3m4's 4
mantissa bits keep the end-to-end rel err ~1.1e-2 < 2e-2.  G is kept
in fp16 (range ~2100 overflows fp8), output stored fp16.

Layout: host feeds Dec pre-transposed (DecT [D, T]) and enc
pre-shuffled to the SBUF tile layout [p, n*d]; host transposes the
fp16 OutT back during the gather.

Schedule per core:
  - all DMAs on the two HWDGE rings (SP + ACT); SWDGE is never used
    (its ~2us completion-semaphore latency stalled the final matmuls).
  - enc loads first in 4 chunks (2 per ring) so the Gram matmuls start
    on the first-landed half; dect follows on both rings.
  - junk warmup matmuls at body start keep PE busy so the HAM clock
    gate releases (1.2 -> 2.4 GHz) as early as possible.
  - final OutT = G @ DecT in 4 x N=512 chunks; PSUM->SBUF copies
    alternate DVE/ACT; stores alternate the two rings.
"""

import os
import sys
from contextlib import ExitStack

import numpy as np

for _p in (
    "/opt/trn_rl_repo",
    "/root/.axon_site",
    "/root/.axon_site/_ro/trn_rl_repo",
    "/root/.axon_site/_ro/pypackages",
):
    if os.path.isdir(_p) and _p not in sys.path:
        sys.path.append(_p)

import concourse.bacc as bacc
import concourse.mybir as mybir
import concourse.tile as tile
from concourse.bass_utils import run_bass_kernel_spmd

B, T, D, P = 8, 2048, 128, 128
NT = T // P  # 16 row tiles of 128

# tunables
MM_DTYPE = "fp8e3"  # "fp8e3" | "fp8e4" | "fp16" (input dtype for enc+dec)
ENC_CHUNKS = 4
DEC_CHUNKS = 2
FINAL_N = 512  # moving-operand width of the final matmul (PSUM bank limit)
WARMUP_MMS = 18  # junk matmuls issued early to trigger the PE HAM clock ramp
OUT_FP16 = True


def _dt(mm_dtype):
    return {
        "fp8e3": mybir.dt.float8e3,
        "fp8e4": mybir.dt.float8e4,
        "fp16": mybir.dt.float16,
        "bf16": mybir.dt.bfloat16,
    }[mm_dtype]


def _build_nc(mm_dtype=None):
    mm_dtype = mm_dtype or MM_DTYPE
    nc = bacc.Bacc("TRN2", target_bir_lowering=False, debug=False)
    f32 = mybir.dt.float32
    fp16 = mybir.dt.float16
    bf16 = mybir.dt.bfloat16
    in_dt = _dt(mm_dtype)

    enc_h = nc.dram_tensor("enc", [P, NT * D], in_dt, kind="ExternalInput")
    dect_h = nc.dram_tensor("dect", [D, T], in_dt, kind="ExternalInput")
    out_dt = fp16 if OUT_FP16 else f32
    out_h = nc.dram_tensor("out", [D, T], out_dt, kind="ExternalOutput")

    enc_v = enc_h.ap().rearrange("p (n d) -> p n d", d=D)
    dect_v = dect_h.ap()
    out_v = out_h.ap()

    with ExitStack() as ctx:
        tc = ctx.enter_context(tile.TileContext(nc))
        singles = ctx.enter_context(tc.tile_pool(name="singles", bufs=1))
        psum = ctx.enter_context(tc.tile_pool(name="psum", bufs=4, space="PSUM"))
        gpsum = ctx.enter_context(tc.tile_pool(name="gpsum", bufs=1, space="PSUM"))

        enc_sb = singles.tile([P, NT, D], in_dt)
        dect_sb = singles.tile([P, T], in_dt)
        out_sb = singles.tile([P, T], out_dt)

        # ---- loads: enc first (2 chunks per ring), then dect ----
        base, rem = divmod(NT, ENC_CHUNKS)
        sizes = [base + (1 if c < rem else 0) for c in range(ENC_CHUNKS)]
        pos = 0
        enc_bounds = []
        for c, sz in enumerate(sizes):
            eng = nc.sync if c % 2 == 0 else nc.scalar
            eng.dma_start(
                out=enc_sb[:, pos : pos + sz, :],
                in_=enc_v[:, pos : pos + sz, :],
            )
            pos += sz
            enc_bounds.append(pos)
        cs = T // DEC_CHUNKS
        for c in range(DEC_CHUNKS):
            eng = nc.sync if c % 2 == 0 else nc.scalar
            eng.dma_start(
                out=dect_sb[:, c * cs : (c + 1) * cs],
                in_=dect_v[:, c * cs : (c + 1) * cs],
            )

        # ---- PE warmup (HAM clock-gate release) ----
        if WARMUP_MMS:
            wsrc = singles.tile([P, P], bf16)
            nc.gpsimd.memset(wsrc[:], 0.0)
            wps = gpsum.tile([P, P], f32, tag="warm")
            for w in range(WARMUP_MMS):
                nc.tensor.matmul(
                    wps[:],
                    lhsT=wsrc[:],
                    rhs=wsrc[:],
                    start=(w == 0),
                    stop=(w == WARMUP_MMS - 1),
                )
            wsink = singles.tile([P, 1], f32)
            nc.vector.tensor_copy(wsink[:], wps[:, :1])

        # ---- Gram matrix: G = sum_i EncTile_i^T @ EncTile_i ----
        g_sb = singles.tile([P, P], fp16)
        g_ps = gpsum.tile([P, P], f32, tag="ga")
        for i in range(NT):
            nc.tensor.matmul(
                g_ps[:],
                lhsT=enc_sb[:, i, :],
                rhs=enc_sb[:, i, :],
                start=(i == 0),
                stop=(i == NT - 1),
            )
        nc.vector.tensor_copy(g_sb[:], g_ps[:])

        # ---- OutT = G @ DecT (G symmetric so lhsT=G is fine) ----
        n_final = T // FINAL_N
        for c in range(n_final):
            op = psum.tile([P, FINAL_N], f32, tag="op")
            rhs = dect_sb[:, c * FINAL_N : (c + 1) * FINAL_N]
            nc.tensor.matmul(op[:], lhsT=g_sb[:], rhs=rhs, start=True, stop=True)
            lo = c * FINAL_N
            if c % 2 == 0:
                nc.vector.tensor_copy(out_sb[:, lo : lo + FINAL_N], op[:])
            else:
                nc.scalar.copy(out_sb[:, lo : lo + FINAL_N], op[:])
            deng = nc.sync if c % 2 == 0 else nc.scalar
            deng.dma_start(
                out=out_v[:, lo : lo + FINAL_N],
                in_=out_sb[:, lo : lo + FINAL_N],
            )

    nc.compile()
    return nc


_NC = {}


def _get_nc(mm_dtype=None):
    mm_dtype = mm_dtype or MM_DTYPE
    if mm_dtype not in _NC:
        _NC[mm_dtype] = _build_nc(mm_dtype)
    return _NC[mm_dtype]


def _np_in_dtype(mm_dtype):
    import ml_dtypes

    return {
        "fp8e3": ml_dtypes.float8_e3m4,
        "fp8e4": ml_dtypes.float8_e4m3,
        "fp16": np.float16,
        "bf16": ml_dtypes.bfloat16,
    }[mm_dtype]


def _run(enc, dec, mm_dtype=None, **kwargs):
    mm_dtype = mm_dtype or MM_DTYPE
    nc = _get_nc(mm_dtype)
    np_dt = _np_in_dtype(mm_dtype)
    in_maps = []
    for b in range(B):
        in_maps.append(
            {
                "enc": np.ascontiguousarray(
                    enc[b].astype(np_dt).reshape(NT, P, D).transpose(1, 0, 2).reshape(P, NT * D)
                ),
                "dect": np.ascontiguousarray(dec[b].T.astype(np_dt)),
            }
        )
    res = run_bass_kernel_spmd(nc, in_maps, core_ids=list(range(B)), **kwargs)
    out = np.stack([res.results[b]["out"].T.astype(np.float32) for b in range(B)], axis=0)
    return np.ascontiguousarray(out), res


def kernel(encoder_hidden_states, decoder_hidden_states):
    enc = np.ascontiguousarray(np.asarray(encoder_hidden_states, dtype=np.float32))
    dec = np.ascontiguousarray(np.asarray(decoder_hidden_states, dtype=np.float32))
    assert enc.shape == (B, T, D) and dec.shape == (B, T, D)
    out, _ = _run(enc, dec)
    return out
